# revision 4
# baseline (speedup 1.0000x reference)
"""Multi-head attention (B=8, C=512, L=2048, H=8, D=64) on 8 TRN2 NeuronCores.

Sharding: pure batch-parallel - core b computes batch b end-to-end (qkv proj,
8 heads of attention, out proj). No collectives.

Per-core layout strategy (v2 - dual-engine softmax):
  - qkv projection with lhsT = w_qkv.T (host-transposed), rhs = x.
  - S^T = K^T Q  (keys on partitions) so the exp output is already the
    transposed P^T needed by the PV matmul, and no max-subtraction is needed
    (scores are ~N(0,1) after the 1/sqrt(D) scale, folded into exp's scale).
  - Heads are processed in pairs (2t, 2t+1) on partition halves 0-63 / 64-127
    of one qkv row-tile. The two K=64 S^T matmuls of a pair run CONCURRENTLY
    in the PE array (row groups 0-1 vs 2-3) writing SEPARATE [128,512] psum
    tiles (st0/st1, one bank each), so each head's exp can start as soon as
    its own half is done - pt0 is ready before PV(h0) issues (no PE stall on
    the exp, unlike the old single [128,1024] exp covering both heads).
  - exp is split across TWO engines: head h0's half always on ScalarE
    (activation Exp, scale=8); head h1's half on VectorE for a tunable subset
    of j-tiles (2-op custom DVE: quartic poly then ^8 - the DVE pipeline is
    capped at 8 ALU ops/instruction so poly+3 squarings cannot fuse). The
    corresponding PV(h1) is deferred one iteration so the in-order PE queue
    never waits on the slower DVE exp. The exp scale is 8 (not 16): scores
    stay within +-6.8 so score/8 is in the quartic's fit range.
  - PV uses lhsT = [V^T | ones] (65 columns): row 64 of the accumulator is
    the softmax denominator, computed for free.
  - V^T is computed directly from X (lhsT = X tiles), V is never materialized.
  - softmax normalization and the output-projection bias-add run mostly on
    GpSimd (Pool), which is otherwise idle, keeping VectorE free for exp.
  - i is processed in 512-wide chunks (outer loop) so each chunk of the
    output projection overlaps the next chunk's attention pass.
"""

import os
import sys

sys.path.insert(0, "/opt/trn_rl_repo")

import numpy as np
import ml_dtypes

import concourse.bass as bass
import concourse.tile as tile
from concourse import bacc, mybir
from concourse import bass_utils

# ---- custom DVE exp: p = poly4(v), then p^8 (v = S/8) ----------------------
# Offloads part of the softmax exp from the (saturated) ScalarE to VectorE.
from concourse.dve_spec import Spec, Src0, C0, C1, C2, One, sq, lower, _has_src1
import concourse.dve_ops as dve_ops
from concourse.dve_ops import DveOp
from concourse.dve_uop import DveOpSpec

# minimax-ish fit of 1+v+v^2(c0+c1 v+c2 v^2) ~ e^v on |v| <= 0.85
# (max rel err 4.2e-4 -> 3.3e-3 after ^8; scores to +-6.8 sigma covered)
EXP_C = (0.50168003, 0.17185385, 0.03959494)


def _register_dve_op(name, spec):
    if name in dve_ops._SUB_OPCODE_FOR_NAME:
        return next(op for op in dve_ops.OPS if op.name == name)
    row = max(dve_ops._SUB_OPCODE_FOR_NAME.values()) + 1
    assert row < 0x20
    dve_ops._SUB_OPCODE_FOR_NAME[name] = row
    shas = {}
    for ver in ("v3", "v4"):
        s = DveOpSpec(
            name=name, opcode=row, uops=lower(spec, ver=ver), rd1_en=_has_src1(spec)
        )
        shas[ver] = s.sha(ver)
    op = DveOp(name, spec, subdim=False, uops_sha=shas)
    dve_ops.OPS.append(op)
    dve_ops.CUSTOM_DVE_SPECS[name] = spec
    return op


def _make_exp_ops():
    t = sq(Src0)
    spec1 = Spec(
        body=(One + Src0) + t * (C0 + C1 * Src0 + C2 * t),
        reference=lambda in0, in1, s0, s1, imm2: (
            1.0 + in0 + in0 * in0 * (s0 + s1 * in0 + imm2 * in0 * in0)
        ).astype(np.float32),
    )
    spec2 = Spec(
        body=sq(sq(sq(Src0))),
        reference=lambda in0, in1, s0, s1, imm2: (in0**8).astype(np.float32),
    )
    return (
        _register_dve_op("EXP8_POLY_ANT", spec1),
        _register_dve_op("POW8_ANT", spec2),
    )


EXP8_POLY, POW8 = _make_exp_ops()

B, C, L = 8, 512, 2048
H, D = 8, 64
HID = H * D  # 512
SCALE = float(D) ** -0.5
BF16 = mybir.dt.bfloat16
F32 = mybir.dt.float32
AF = mybir.ActivationFunctionType
NCORES = 8

NT = C // 128  # 4 channel tiles
NL = L // 512  # 4 l-chunks of 512
NJ = L // 128  # 16 key tiles

# j-tiles whose h1-half exp runs on VectorE instead of ScalarE.
# jt 15 must stay on ScalarE (its PV cannot defer past the pair boundary).
_DVE_DEFAULT = "1,3,5,7,9,11,13,6,10"
DVE_JTS = frozenset(
    int(x)
    for x in os.environ.get("KERNEL_DVE_JTS", _DVE_DEFAULT).split(",")
    if x != ""
) - {NJ - 1}


def build_kernel(tc, out_d, x_d, wqkvT_d, woutT_d, bias_d):
    nc = tc.nc
    from contextlib import ExitStack

    ctx = ExitStack()
    pers = ctx.enter_context(tc.tile_pool(name="pers", bufs=1))
    ptp = ctx.enter_context(tc.tile_pool(name="ptp", bufs=3))
    scrp = ctx.enter_context(tc.tile_pool(name="scrp", bufs=2))
    ytp = ctx.enter_context(tc.tile_pool(name="ytp", bufs=3))
    smp = ctx.enter_context(tc.tile_pool(name="smp", bufs=3))
    st0p = ctx.enter_context(tc.tile_pool(name="st0p", bufs=2, space="PSUM"))
    st1p = ctx.enter_context(tc.tile_pool(name="st1p", bufs=3, space="PSUM"))
    otp = ctx.enter_context(tc.tile_pool(name="otp", bufs=1, space="PSUM"))
    qkp = ctx.enter_context(tc.tile_pool(name="qkp", bufs=1, space="PSUM"))

    # ---- persistent SBUF tensors ----
    x_sb = [pers.tile([128, L], BF16, tag=f"x{c}", name=f"x{c}") for c in range(NT)]
    wq_sb = [
        pers.tile([128, 3 * HID], BF16, tag=f"wq{c}", name=f"wq{c}") for c in range(NT)
    ]
    wo_sb = [pers.tile([128, C], BF16, tag=f"wo{c}", name=f"wo{c}") for c in range(NT)]
    bias_sb = [
        pers.tile([128, 1], F32, tag=f"bias{c}", name=f"bias{c}") for c in range(NT)
    ]
    q_sb = [pers.tile([128, L], BF16, tag=f"q{t}", name=f"q{t}") for t in range(NT)]
    k_sb = [pers.tile([128, L], BF16, tag=f"k{t}", name=f"k{t}") for t in range(NT)]
    vt1 = [
        pers.tile([128, H * 65], BF16, tag=f"vt{j}", name=f"vt{j}") for j in range(NJ)
    ]
    o2 = [pers.tile([128, L], BF16, tag=f"o2_{c}", name=f"o2_{c}") for c in range(NT)]

    # ---- input DMAs, two waves on three trigger queues (Sync/Scalar/GpSimd).
    # Wave 1 is exactly what the first q/k projection groups and first V^T
    # tiles need (~0.8MB) so the first exp is not gated by the full 3.5MB
    # input load; wave 2 streams the rest behind it. ----
    for c in range(NT):
        r = slice(128 * c, 128 * (c + 1))
        nc.sync.dma_start(x_sb[c][:, 0:512], x_d[r, 0:512])
        nc.scalar.dma_start(wq_sb[c][:, 0:128], wqkvT_d[r, 0:128])
        nc.gpsimd.dma_start(wq_sb[c][:, 512:640], wqkvT_d[r, 512:640])
    for c in range(NT):
        r = slice(128 * c, 128 * (c + 1))
        nc.gpsimd.dma_start(wq_sb[c][:, 1024:1536], wqkvT_d[r, 1024:1536])
    for c in range(NT):
        r = slice(128 * c, 128 * (c + 1))
        nc.sync.dma_start(x_sb[c][:, 512:1024], x_d[r, 512:1024])
        nc.sync.dma_start(x_sb[c][:, 1024:1536], x_d[r, 1024:1536])
        nc.sync.dma_start(x_sb[c][:, 1536:2048], x_d[r, 1536:2048])
        nc.scalar.dma_start(wq_sb[c][:, 128:512], wqkvT_d[r, 128:512])
        nc.scalar.dma_start(wq_sb[c][:, 640:1024], wqkvT_d[r, 640:1024])

    # ---- PE warm-up: dummy matmuls during the input-DMA window so the HAM
    # clock gate opens (1.2 -> 2.4 GHz) before the real work arrives. The
    # chain ends in a DMA to an internal DRAM scratch so DCE keeps it. ----
    warm_scratch = nc.dram_tensor("warm_scratch", [128, 512], F32)
    warm_sb = pers.tile([128, 512], BF16, tag="warm", name="warm_sb")
    warm_out = pers.tile([128, 512], F32, tag="warmo", name="warm_out")
    nc.vector.memset(warm_sb[:, :], 0.001)
    wps = qkp.tile([128, 512], F32, tag="qkp", name="warm_ps")
    for w in range(18):
        nc.tensor.matmul(
            wps[:, :], lhsT=warm_sb[:, 0:128], rhs=warm_sb[:, :],
            start=True, stop=True,
        )
    nc.vector.tensor_copy(warm_out[:, :], wps[:, :])
    nc.sync.dma_start(warm_scratch.ap()[:, :], warm_out[:, :])

    def emit_qk_group(t, kind, n):
        """One projection psum group: q (kind=0) or k (kind=1) rows
        128t..128t+128 (heads 2t, 2t+1), l-chunk n. Lands directly in
        q_sb/k_sb (head 2t on partitions 0-63, head 2t+1 on 64-127)."""
        dst = (q_sb, k_sb)[kind][t]
        ocol = kind * HID + 128 * t
        ps = qkp.tile([128, 512], F32, tag="qkp", name=f"qk_ps_{kind}_{t}_{n}")
        for c in range(NT):
            nc.tensor.matmul(
                ps[:, :],
                lhsT=wq_sb[c][:, ocol : ocol + 128],
                rhs=x_sb[c][:, 512 * n : 512 * (n + 1)],
                start=(c == 0),
                stop=(c == NT - 1),
            )
        nc.vector.tensor_copy(dst[:, 512 * n : 512 * (n + 1)], ps[:, :])

    def emit_vt(jt):
        """V^T tile for key-block jt: [128 keys, 8 heads x (64 dims + ones)]."""
        ps = qkp.tile([128, 512], F32, tag="qkp", name=f"vt_ps_{jt}")
        for c in range(NT):
            nc.tensor.matmul(
                ps[:, :],
                lhsT=x_sb[c][:, 128 * jt : 128 * (jt + 1)],
                rhs=wq_sb[c][:, 2 * HID : 3 * HID],
                start=(c == 0),
                stop=(c == NT - 1),
            )
        vv = vt1[jt].rearrange("p (h e) -> p h e", e=65)
        nc.vector.tensor_copy(vv[:, :, 0:64], ps.rearrange("p (h d) -> p h d", d=64))
        nc.vector.memset(vv[:, :, 64:65], 1.0)

    def emit_st_for(t, ic, jt):
        """S^T for head pair t, i-chunk ic, key block jt: two K=64 matmuls
        (PE row groups 0-1 / 2-3, concurrent) into separate 1-bank psum
        tiles so each head's exp is gated only by its own half."""
        islice = slice(512 * ic, 512 * ic + 512)
        jslice = slice(128 * jt, 128 * (jt + 1))
        s0 = st0p.tile([128, 512], F32, tag="st0", name=f"st0_{t}_{ic}_{jt}")
        s1 = st1p.tile([128, 512], F32, tag="st1", name=f"st1_{t}_{ic}_{jt}")
        nc.tensor.matmul(
            s0[:, :], lhsT=k_sb[t][0:64, jslice], rhs=q_sb[t][0:64, islice],
            start=True, stop=True,
        )
        nc.tensor.matmul(
            s1[:, :], lhsT=k_sb[t][64:128, jslice], rhs=q_sb[t][64:128, islice],
            start=True, stop=True,
        )
        return (s0, s1)

    def emit_pair(t, ic, interleave, vt_jit=False, first_st=None, next_ti=None,
                  dve_jts=frozenset()):
        """Attention for head pair (2t, 2t+1), i-chunk ic (512 queries).
        `interleave` closures emit independent PE work into the loop; with
        vt_jit the V^T tiles are emitted just-in-time ahead of the PV that
        first needs them. `first_st` is this pair's S^T(0) tiles if the
        previous pair already emitted them (cross-pair pipelining); if
        `next_ti` is given, the NEXT pair's S^T(0) is emitted BEFORE the last
        PVs, so at the boundary the exps never wait behind PV(15). h1-half
        exps for jt in `dve_jts` run on VectorE (2-op poly^8) with their PV
        deferred one iteration. Returns the next pair's S^T(0) tiles."""
        h0, h1 = 2 * t, 2 * t + 1
        ib = 512 * ic
        islice = slice(ib, ib + 512)
        ot0 = otp.tile([65, 512], F32, tag="ot0", name=f"ot0_{t}_{ic}")
        ot1 = otp.tile([65, 512], F32, tag="ot1", name=f"ot1_{t}_{ic}")

        pv_cnt = [0, 0]

        def emit_pv(hx, jt, pt):
            pv_cnt[hx] += 1
            ot = (ot0, ot1)[hx]
            h = (h0, h1)[hx]
            vt = vt1[jt]
            nc.tensor.matmul(
                ot[:, :], lhsT=vt[:, 65 * h : 65 * h + 65], rhs=pt[:, :],
                start=(jt == 0), stop=(pv_cnt[hx] == NJ),
            )

        slot = 0
        deferred = []
        next_first = None
        sts = {0: first_st if first_st is not None else emit_st_for(t, ic, 0)}
        for jt in range(NJ):
            s0, s1 = sts.pop(jt)
            # h0 exp on ScalarE immediately - gated only by its own S^T half,
            # so pt0 is ready before the PE reaches PV(h0) this iteration.
            pt0 = ptp.tile([128, 512], BF16, tag="pt0", name=f"pt0_{t}_{ic}_{jt}")
            nc.scalar.activation(pt0[:, :], s0[:, :], AF.Exp, scale=8.0)
            use_dve = jt in dve_jts
            pt1 = ptp.tile([128, 512], BF16, tag="pt1", name=f"pt1_{t}_{ic}_{jt}")
            if use_dve:
                # VectorE exp: exp(8v) = (poly4(v))^8 - q weights are
                # host-prescaled by SCALE/8 so the matmul emits v directly
                p1 = scrp.tile([128, 512], F32, tag="p1", name=f"p1_{t}_{ic}_{jt}")
                nc.vector._custom_dve(
                    EXP8_POLY, out=p1[:, :], in0=s1[:, :],
                    s0=EXP_C[0], s1=EXP_C[1], imm2=EXP_C[2],
                )
                nc.vector._custom_dve(POW8, out=pt1[:, :], in0=p1[:, :])
            else:
                nc.scalar.activation(pt1[:, :], s1[:, :], AF.Exp, scale=8.0)
            if jt + 1 < NJ:
                sts[jt + 1] = emit_st_for(t, ic, jt + 1)
            elif next_ti is not None:
                # cross-pair: next pair's S^T(0) goes ahead of this pair's
                # last PVs in the PE stream
                next_first = emit_st_for(next_ti[0], next_ti[1], 0)
            # V^T tiles emitted in-loop so they never gate the first exp;
            # >=2-iteration lead keeps their DVE copies off PV's critical path
            if vt_jit:
                if jt == 0:
                    emit_vt(0)
                    emit_vt(1)
                    emit_vt(2)
                elif jt + 2 < NJ:
                    emit_vt(jt + 2)
            # a DVE-produced pt1 arrives ~1us later than a ScalarE one; its
            # PV would head-of-line-block the in-order PE queue, so defer it
            # one iteration (PSUM accumulation order is preserved: deferred
            # jt-1 flushes before this iteration's h1 PV).
            while deferred and deferred[0][0] <= jt - 1:
                emit_pv(1, *deferred.pop(0))
            emit_pv(0, jt, pt0)
            if use_dve:
                deferred.append((jt, pt1))
            else:
                emit_pv(1, jt, pt1)
            # fill PE slack with independent work, spread across the loop
            target = ((jt + 1) * len(interleave) + 11) // 12
            while slot < min(target, len(interleave)):
                interleave[slot]()
                slot += 1
        for djt, dpt in deferred:
            emit_pv(1, djt, dpt)
        # softmax normalization: divide rows 0-63 by the ones-row (64).
        # ot0 is evacuated on VectorE (fast, frees the psum bank for the next
        # pair's first PV); everything else runs on the otherwise-idle Pool
        # engine. reciprocal_approx_fast mis-reads non-zero partition offsets
        # on silicon, so the denominator row is staged to partition 0 first.
        o2u0 = smp.tile([65, 512], F32, tag="o2u0", name=f"o2u_{h0}_{ic}")
        nc.vector.tensor_copy(o2u0[:, :], ot0[:, :])
        den0 = smp.tile([1, 512], F32, tag="den0", name=f"den_{h0}_{ic}")
        nc.gpsimd.tensor_copy(den0[:, :], o2u0[64:65, :])
        o2u1 = smp.tile([65, 512], F32, tag="o2u1", name=f"o2u_{h1}_{ic}")
        nc.vector.tensor_copy(o2u1[:, :], ot1[:, :])
        den1 = smp.tile([1, 512], F32, tag="den1", name=f"den_{h1}_{ic}")
        nc.gpsimd.tensor_copy(den1[:, :], o2u1[64:65, :])
        rec0 = smp.tile([1, 512], F32, tag="rec0", name=f"rec_{h0}_{ic}")
        nc.vector.reciprocal_approx_fast(rec0[:, :], den0[:, :])
        rec1 = smp.tile([1, 512], F32, tag="rec1", name=f"rec_{h1}_{ic}")
        nc.vector.reciprocal_approx_fast(rec1[:, :], den1[:, :])
        rb0 = smp.tile([64, 512], F32, tag="rb0", name=f"rb_{h0}_{ic}")
        nc.gpsimd.partition_broadcast(rb0[:, :], rec0[:, :])
        nc.gpsimd.tensor_mul(o2[t][0:64, islice], o2u0[0:64, :], rb0[:, :])
        rb1 = smp.tile([64, 512], F32, tag="rb1", name=f"rb_{h1}_{ic}")
        nc.gpsimd.partition_broadcast(rb1[:, :], rec1[:, :])
        nc.gpsimd.tensor_mul(o2[t][64:128, islice], o2u1[0:64, :], rb1[:, :])
        return next_first

    held_proj = {}

    def emit_proj_group(o, n, c_lo=0):
        if c_lo == 0:
            ps = qkp.tile([128, 512], F32, tag="qkp", name=f"y_ps_{o}_{n}")
        else:
            ps = held_proj.pop((o, n))
        for c in range(c_lo, NT):
            nc.tensor.matmul(
                ps[:, :],
                lhsT=wo_sb[c][:, 128 * o : 128 * (o + 1)],
                rhs=o2[c][:, 512 * n : 512 * (n + 1)],
                start=(c == 0),
                stop=(c == NT - 1),
            )
        yt = ytp.tile([128, 512], F32, tag="yt", name=f"yt_{o}_{n}")
        nc.vector.tensor_scalar_add(yt[:, :], ps[:, :], bias_sb[o][:, 0:1])
        nc.sync.dma_start(
            out_d[128 * o : 128 * (o + 1), 512 * n : 512 * (n + 1)], yt[:, :]
        )

    def emit_proj_partial(o, n):
        """First 3 channel-tiles of proj group (o, n); the psum tile is held
        and finished by emit_proj_group(o, n, c_lo=3) once the last pair's
        output is ready."""
        ps = qkp.tile([128, 512], F32, tag="qkp", name=f"y_ps_{o}_{n}")
        for c in range(3):
            nc.tensor.matmul(
                ps[:, :],
                lhsT=wo_sb[c][:, 128 * o : 128 * (o + 1)],
                rhs=o2[c][:, 512 * n : 512 * (n + 1)],
                start=(c == 0),
                stop=False,
            )
        held_proj[(o, n)] = ps

    # ---- emission schedule ----
    # pair 0's q (chunk 0) + full k projected up front; everything else is
    # interleaved just-in-time into earlier attention loops.
    emit_qk_group(0, 0, 0)
    emit_qk_group(0, 1, 0)

    # wo/bias loads off the critical startup path
    for c in range(NT):
        r = slice(128 * c, 128 * (c + 1))
        nc.sync.dma_start(wo_sb[c][:, :], woutT_d[r, :])
        nc.sync.dma_start(bias_sb[c][:, :], bias_d[r, :])

    def kg(t, n):
        return lambda: emit_qk_group(t, 1, n)

    def qg(t, n):
        return lambda: emit_qk_group(t, 0, n)

    def pj(o, n):
        return lambda: emit_proj_group(o, n)

    # pair t's q chunk for pass ic must be emitted BEFORE its (ic, t) loop
    # (the PE executes in order - a dependency later in its own stream would
    # deadlock). q chunks for pass ic+1 therefore fire during pass ic, and
    # proj chunk n fires during pass n+1.
    # later k chunks of a pair may fire early inside that pair's OWN loop
    # (k block n is first read at jt=4n, well after the interleave slot).
    inter = {
        # k0 chunks 1-3 fire inside pair 0's own loop (k block n is first
        # read at jt=4n, after its interleave slot); vt tiles are JIT
        (0, 0): [kg(0, 1), kg(0, 2), kg(0, 3), qg(1, 0), kg(1, 0)],
        (0, 1): [kg(1, 1), kg(1, 2), kg(1, 3), qg(2, 0), kg(2, 0)],
        (0, 2): [kg(2, 1), kg(2, 2), kg(2, 3), qg(3, 0), kg(3, 0), qg(0, 1)],
        (0, 3): [kg(3, 1), kg(3, 2), kg(3, 3), qg(1, 1), qg(2, 1), qg(3, 1)],
        (1, 0): [qg(0, 2), pj(0, 0)],
        (1, 1): [qg(1, 2), pj(1, 0)],
        (1, 2): [qg(2, 2), pj(2, 0)],
        (1, 3): [qg(3, 2), pj(3, 0)],
        (2, 0): [qg(0, 3), pj(0, 1)],
        (2, 1): [qg(1, 3), pj(1, 1)],
        (2, 2): [qg(2, 3), pj(2, 1)],
        (2, 3): [qg(3, 3), pj(3, 1)],
        (3, 0): [pj(0, 2)],
        (3, 1): [pj(1, 2)],
        (3, 2): [pj(2, 2), pj(3, 2)],
        (3, 3): [
            lambda: emit_proj_partial(0, 3),
            lambda: emit_proj_partial(1, 3),
        ],
    }
    seq = [(ic, t) for ic in range(4) for t in range(NT)]
    pending_st = None
    for i, (ic, t) in enumerate(seq):
        nxt = seq[i + 1] if i + 1 < len(seq) else None
        # no DVE offload in the very first pair (its DVE queue is busy with
        # JIT V^T evacuations) and none on jt 15 (enforced in DVE_JTS)
        dj = frozenset() if (ic == 0 and t == 0) else DVE_JTS
        pending_st = emit_pair(
            t, ic, inter.get((ic, t), []),
            vt_jit=(ic == 0 and t == 0),
            first_st=pending_st,
            next_ti=(nxt[1], nxt[0]) if nxt else None,
            dve_jts=dj,
        )
    emit_proj_group(0, 3, c_lo=3)
    emit_proj_group(1, 3, c_lo=3)
    emit_proj_group(2, 3)
    emit_proj_group(3, 3)
    ctx.close()


_COMPILED = None


def _get_compiled():
    global _COMPILED
    if _COMPILED is None:
        nc = bacc.Bacc(
            "TRN2", target_bir_lowering=False, debug=False, num_devices=NCORES
        )
        x_d = nc.dram_tensor("x", [C, L], BF16, kind="ExternalInput").ap()
        wqkvT_d = nc.dram_tensor("wqkvT", [C, 3 * HID], BF16, kind="ExternalInput").ap()
        woutT_d = nc.dram_tensor("woutT", [HID, C], BF16, kind="ExternalInput").ap()
        bias_d = nc.dram_tensor("bias", [C, 1], F32, kind="ExternalInput").ap()
        out_d = nc.dram_tensor("out", [C, L], F32, kind="ExternalOutput").ap()
        with tile.TileContext(nc) as tc:
            build_kernel(tc, out_d, x_d, wqkvT_d, woutT_d, bias_d)
        nc.compile()
        _COMPILED = nc
    return _COMPILED


def make_in_maps(x, w_qkv, w_out, b_out):
    xb = np.asarray(x, dtype=np.float32).astype(ml_dtypes.bfloat16)
    wq_f = np.asarray(w_qkv, dtype=np.float32).T.copy()
    wq_f[:, 0:HID] *= SCALE / 8.0  # exp scale folded into the q projection
    wqkvT = np.ascontiguousarray(wq_f.astype(ml_dtypes.bfloat16))
    woutT = np.ascontiguousarray(
        np.asarray(w_out, dtype=np.float32).T.astype(ml_dtypes.bfloat16)
    )
    bias = np.ascontiguousarray(np.asarray(b_out, dtype=np.float32).reshape(C, 1))
    return [
        {
            "x": np.ascontiguousarray(xb[b]),
            "wqkvT": wqkvT,
            "woutT": woutT,
            "bias": bias,
        }
        for b in range(B)
    ]


LAST_RESULTS = None


def _install_ntff_hook():
    """Provide antenv.axon_hooks (absent from this image) so trace=True works."""
    import types

    try:
        from antenv.axon_hooks import get_axon_ntff_profile_hook  # noqa: F401

        return
    except ImportError:
        pass
    sys.path.insert(0, "/root/.axon_site")
    from trn_agent_boot.trn_boot import _ntff_profile_via_ctypes

    hook = _ntff_profile_via_ctypes("/opt/axon/libaxon_pjrt.so")
    import antenv

    mod = types.ModuleType("antenv.axon_hooks")
    mod._hook = hook
    mod.get_axon_ntff_profile_hook = lambda: mod._hook
    mod.set_axon_ntff_profile_hook = lambda h: setattr(mod, "_hook", h)
    sys.modules["antenv.axon_hooks"] = mod
    antenv.axon_hooks = mod
    # artifact upload has no egress in this container - make it a no-op
    bass_utils.upload_artifacts = lambda tmpdir: tmpdir


def kernel(x, w_qkv, w_out, b_out):
    global LAST_RESULTS
    nc = _get_compiled()
    in_maps = make_in_maps(x, w_qkv, w_out, b_out)
    trace = bool(int(os.environ.get("KERNEL_TRACE", "0")))
    if trace:
        _install_ntff_hook()
    res = bass_utils.run_bass_kernel_spmd(
        nc, in_maps, core_ids=list(range(NCORES)), trace=trace
    )
    LAST_RESULTS = res
    out = np.stack([np.asarray(res.results[b]["out"]) for b in range(B)])
    return out.astype(np.float32)
